# revision 14
# baseline (speedup 1.0000x reference)
"""Trainium2 Bass kernel for nn_AutoregressiveDecoder.

2-layer GRU (H=512) over T=1024 steps, teacher-forced, + output heads.
Sharding: data-parallel over batch B=128 across 8 cores (16 rows each).

The toolchain can't compile hardware loops (walrus rejects register-offset
control flow), so the recurrence is split into 64 straight-line 16-step
chunk programs executed back-to-back; h-state, weights, and the h1 history
stay device-resident between launches (one jitted shard_map wrapper around
the bass custom-call, built once per program).

Per-core layout: PE matmuls keep weights stationary (lhsT = W tile
[K=128, M=128 gate dims], rhs = state [K=128, N=16 batch]) so gate outputs
land gates-on-partitions -> [128, 64/128] slab elementwise on DVE/ACT, and
h' comes out exactly in next step's rhs layout (no transposes). Heads run
token-major in bulk afterwards.
"""
import sys

if "/opt/trn_rl_repo" not in sys.path:
    sys.path.insert(0, "/opt/trn_rl_repo")

import numpy as np

B, T, Z, H, F, NK, L = 128, 1024, 128, 512, 128, 2, 2
NCORES = 8
NB = B // NCORES            # 16 batch rows per core
MT = (3 * H) // 128         # 12 gate M-tiles
KH = H // 128               # 4 contraction tiles over H
TC = 16                     # steps per chunk program
MIN_LS, MAX_LS = -5.0, 2.0
F32 = np.float32


# ---------------- host-side prep (movement/reshape only) ----------------

def _wtiles(w, kt):
    o, i = w.shape
    assert i == kt * 128 and o % 128 == 0
    return np.ascontiguousarray(
        w.reshape(o // 128, 128, kt, 128).transpose(2, 0, 3, 1)).astype(F32)


def _bslab(b, tiles):
    a = b.reshape(-1, 128)[list(tiles)]
    a = np.ascontiguousarray(a.T)[:, :, None]
    return np.ascontiguousarray(
        np.broadcast_to(a, (128, len(tiles), 16)).reshape(128, len(tiles) * 16)
    ).astype(F32)


def _prep_core(iv, c, T_):
    sl = slice(c * NB, (c + 1) * NB)
    x, z, f = iv["x"][sl, :T_], iv["z"][sl], iv["f"][sl, :T_]
    st = np.broadcast_to(iv["start_token"], (NB, 1, F))
    x_prev = np.concatenate([st, x[:, :-1]], axis=1)
    xT = np.ascontiguousarray(x_prev.transpose(1, 2, 0)).reshape(T_ * F, NB)
    wih0 = iv["W_ih0"]
    m = {
        "xT": xT.astype(F32),
        "zT": np.ascontiguousarray(z.T).astype(F32),
        "wxT": _wtiles(wih0[:, :F], 1)[0],
        "wzT": _wtiles(wih0[:, F:], 1)[0],
        "whh0T": _wtiles(iv["W_hh0"], KH),
        "wih1T": _wtiles(iv["W_ih1"], KH),
        "whh1T": _wtiles(iv["W_hh1"], KH),
        "fcT": _wtiles(iv["fc_init_w"], 1)[0],
        "bih0_s": _bslab(iv["b_ih0"], range(12)),
        "bhh0_ru_s": _bslab(iv["b_hh0"], range(8)),
        "bhh0_n_s": _bslab(iv["b_hh0"], range(8, 12)),
        "bih1_ru_s": _bslab(iv["b_ih1"], range(8)),
        "bih1_n_s": _bslab(iv["b_ih1"], range(8, 12)),
        "bhh1_ru_s": _bslab(iv["b_hh1"], range(8)),
        "bhh1_n_s": _bslab(iv["b_hh1"], range(8, 12)),
        "fcb_s": _bslab(iv["fc_init_b"], range(8)),
    }
    lw = iv["load_w"].reshape(F, NK, H)
    wcat = np.concatenate([iv["mu_w"], iv["ls_w"], lw[:, 0], lw[:, 1]], 0)
    m["hW"] = np.ascontiguousarray(wcat.T).reshape(KH, 128, 512).astype(F32)
    m["mub_s"] = np.broadcast_to(iv["mu_b"], (128, F)).astype(F32).copy()
    m["lsb_s"] = np.broadcast_to(iv["ls_b"], (128, F)).astype(F32).copy()
    lb = iv["load_b"].reshape(F, NK)
    lbd = np.concatenate([lb[:, 0], lb[:, 1]])
    m["loadb_s"] = np.broadcast_to(lbd, (128, 256)).astype(F32).copy()
    ntt = T_ * NB // 128
    fh = np.ascontiguousarray(f.transpose(1, 0, 2)).reshape(ntt, 128, NK)
    m["fh"] = fh.astype(F32)
    return m


# ---------------- bass program builders ----------------

def _mk_nc():
    from concourse import bacc
    return bacc.Bacc("TRN2", target_bir_lowering=False, debug=False,
                     num_devices=NCORES)


def _din(nc, name, shape, dt):
    return nc.dram_tensor(name, list(shape), dt, kind="ExternalInput").ap()


def _dout(nc, name, shape, dt):
    return nc.dram_tensor(name, list(shape), dt, kind="ExternalOutput").ap()


def build_init(nc):
    """h_init = tanh(z @ fc^T + b); gizb slabs; b1ru slab."""
    import concourse.mybir as mybir
    from concourse.tile import TileContext
    dt = mybir.dt.float32
    AF = mybir.ActivationFunctionType

    zT = _din(nc, "zT", (Z, NB), dt)
    fcT = _din(nc, "fcT", (8, 128, 128), dt)
    wzT = _din(nc, "wzT", (MT, 128, 128), dt)
    fcb_s = _din(nc, "fcb_s", (128, 128), dt)
    bih0_s = _din(nc, "bih0_s", (128, 192), dt)
    bhh0_ru_s = _din(nc, "bhh0_ru_s", (128, 128), dt)
    bih1_ru_s = _din(nc, "bih1_ru_s", (128, 128), dt)
    bhh1_ru_s = _din(nc, "bhh1_ru_s", (128, 128), dt)

    h01_d = _dout(nc, "h01_d", (128, 128), dt)
    gizb_ru_d = _dout(nc, "gizb_ru_d", (128, 128), dt)
    gizb_n_d = _dout(nc, "gizb_n_d", (128, 64), dt)
    b1ru_d = _dout(nc, "b1ru_d", (128, 128), dt)

    with TileContext(nc) as tc:
        with tc.tile_pool(name="sb", bufs=1) as cp, \
             tc.tile_pool(name="ps", bufs=2, space="PSUM") as pp:
            def csb(src, shape, tag):
                t = cp.tile(list(shape), dt, tag=tag)
                nc.sync.dma_start(out=t, in_=src)
                return t

            z_t = csb(zT, (128, NB), "zt")
            fc_sb = csb(fcT.rearrange("m p c -> p m c"), (128, 8, 128), "fc")
            wz_sb = csb(wzT.rearrange("m p c -> p m c"), (128, MT, 128), "wz")
            fcb = csb(fcb_s, (128, 128), "fcb")
            bih0 = csb(bih0_s, (128, 192), "bih0")
            bhh0ru = csb(bhh0_ru_s, (128, 128), "bhh0ru")
            bih1ru = csb(bih1_ru_s, (128, 128), "bih1ru")
            bhh1ru = csb(bhh1_ru_s, (128, 128), "bhh1ru")

            giz_ps = pp.tile([128, 192], dt, tag="ps")
            for m in range(MT):
                nc.tensor.matmul(giz_ps[:, 16 * m:16 * m + 16],
                                 wz_sb[:, m, :], z_t, start=True, stop=True)
            tmp = cp.tile([128, 128], dt, tag="tmp")
            nc.vector.tensor_add(tmp, bih0[:, 0:128], bhh0ru)
            gzr = cp.tile([128, 128], dt, tag="gzr")
            nc.vector.tensor_add(gzr, giz_ps[:, 0:128], tmp)
            nc.sync.dma_start(out=gizb_ru_d, in_=gzr)
            gzn = cp.tile([128, 64], dt, tag="gzn")
            nc.vector.tensor_add(gzn, giz_ps[:, 128:192], bih0[:, 128:192])
            nc.sync.dma_start(out=gizb_n_d, in_=gzn)
            b1r = cp.tile([128, 128], dt, tag="b1r")
            nc.vector.tensor_add(b1r, bih1ru, bhh1ru)
            nc.sync.dma_start(out=b1ru_d, in_=b1r)

            fc_ps = pp.tile([128, 128], dt, tag="ps")
            for m in range(8):
                nc.tensor.matmul(fc_ps[:, 16 * m:16 * m + 16],
                                 fc_sb[:, m, :], z_t, start=True, stop=True)
            hs = cp.tile([128, 128], dt, tag="hs")
            nc.vector.tensor_add(hs, fc_ps, fcb)
            h01 = cp.tile([128, 128], dt, tag="h01")
            nc.scalar.activation(h01, hs, AF.Tanh)
            nc.sync.dma_start(out=h01_d, in_=h01)
    return nc


def build_chunk(nc):
    """TC GRU steps: h_in -> h_out, scr_c = h1 history [TC*128, 64]."""
    import concourse.mybir as mybir
    from concourse.bass import ds
    from concourse.tile import TileContext
    dt = mybir.dt.float32
    AF = mybir.ActivationFunctionType

    h_in = _din(nc, "h_in", (128, 128), dt)
    x_c = _din(nc, "x_c", (TC * 128, NB), dt)
    gizb_ru_d = _din(nc, "gizb_ru_d", (128, 128), dt)
    gizb_n_d = _din(nc, "gizb_n_d", (128, 64), dt)
    b1ru_d = _din(nc, "b1ru_d", (128, 128), dt)
    bih1_n_s = _din(nc, "bih1_n_s", (128, 64), dt)
    bhh0_n_s = _din(nc, "bhh0_n_s", (128, 64), dt)
    bhh1_n_s = _din(nc, "bhh1_n_s", (128, 64), dt)
    wxT = _din(nc, "wxT", (MT, 128, 128), dt)
    whh0T = _din(nc, "whh0T", (KH, MT, 128, 128), dt)
    wih1T = _din(nc, "wih1T", (KH, MT, 128, 128), dt)
    whh1T = _din(nc, "whh1T", (KH, MT, 128, 128), dt)

    h_out = _dout(nc, "h_out", (128, 128), dt)
    scr_c = _dout(nc, "scr_c", (TC * 128, 64), dt)

    with TileContext(nc) as tc:
        with tc.tile_pool(name="const", bufs=1) as cp, \
             tc.tile_pool(name="work", bufs=3) as wp, \
             tc.tile_pool(name="xin", bufs=4) as xp, \
             tc.tile_pool(name="psA", bufs=2, space="PSUM") as pa, \
             tc.tile_pool(name="psB", bufs=2, space="PSUM") as pb, \
             tc.tile_pool(name="psC", bufs=2, space="PSUM") as pc:

            def csb(src, shape, tag):
                t = cp.tile(list(shape), dt, tag=tag)
                nc.sync.dma_start(out=t, in_=src)
                return t

            wx_sb = csb(wxT.rearrange("m p c -> p m c"), (128, MT, 128), "wx")
            whh0_sb = csb(whh0T.rearrange("k m p c -> p k m c"),
                          (128, KH, MT, 128), "whh0")
            wih1_sb = csb(wih1T.rearrange("k m p c -> p k m c"),
                          (128, KH, MT, 128), "wih1")
            whh1_sb = csb(whh1T.rearrange("k m p c -> p k m c"),
                          (128, KH, MT, 128), "whh1")
            gizb_ru = csb(gizb_ru_d, (128, 128), "gizbru")
            gizb_n = csb(gizb_n_d, (128, 64), "gizbn")
            b1ru = csb(b1ru_d, (128, 128), "b1ru")
            bih1n = csb(bih1_n_s, (128, 64), "bih1n")
            bhh0n = csb(bhh0_n_s, (128, 64), "bhh0n")
            bhh1n = csb(bhh1_n_s, (128, 64), "bhh1n")
            h01 = csb(h_in, (128, 128), "h01")

            for s in range(TC):
                x_t = xp.tile([128, NB], dt, tag="x_t")
                nc.sync.dma_start(out=x_t, in_=x_c[ds(s * 128, 128)])

                ps_a = pa.tile([128, 192], dt, tag="ps_a")
                ps_b = pb.tile([128, 64], dt, tag="ps_b")
                ps_c = pc.tile([128, 256], dt, tag="ps_c")

                # layer0
                for m in range(8):
                    nc.tensor.matmul(ps_a[:, 16 * m:16 * m + 16],
                                     wx_sb[:, m, :], x_t,
                                     start=True, stop=False)
                    for k in range(KH):
                        nc.tensor.matmul(ps_a[:, 16 * m:16 * m + 16],
                                         whh0_sb[:, k, m, :],
                                         h01[:, 16 * k:16 * k + 16],
                                         start=False, stop=(k == KH - 1))
                for m in range(8, MT):
                    nc.tensor.matmul(ps_a[:, 16 * m:16 * m + 16],
                                     wx_sb[:, m, :], x_t,
                                     start=True, stop=True)
                for m in range(8, MT):
                    mm = m - 8
                    for k in range(KH):
                        nc.tensor.matmul(ps_b[:, 16 * mm:16 * mm + 16],
                                         whh0_sb[:, k, m, :],
                                         h01[:, 16 * k:16 * k + 16],
                                         start=(k == 0), stop=(k == KH - 1))

                s_ru = wp.tile([128, 128], dt, tag="s_ru")
                nc.vector.tensor_add(s_ru, ps_a[:, 0:128], gizb_ru)
                ru0 = wp.tile([128, 128], dt, tag="ru0")
                nc.scalar.activation(ru0, s_ru, AF.Sigmoid)
                hn0 = wp.tile([128, 64], dt, tag="hn0")
                nc.vector.tensor_add(hn0, ps_b, bhh0n)
                rhn0 = wp.tile([128, 64], dt, tag="rhn0")
                nc.vector.tensor_mul(rhn0, ru0[:, 0:64], hn0)
                s_n = wp.tile([128, 64], dt, tag="s_n")
                nc.vector.tensor_add(s_n, ps_a[:, 128:192], gizb_n)
                nin0 = wp.tile([128, 64], dt, tag="nin0")
                nc.vector.tensor_add(nin0, s_n, rhn0)
                n0 = wp.tile([128, 64], dt, tag="n0")
                nc.scalar.activation(n0, nin0, AF.Tanh)
                d0 = wp.tile([128, 64], dt, tag="d0")
                nc.vector.tensor_sub(d0, h01[:, 0:64], n0)
                e0 = wp.tile([128, 64], dt, tag="e0")
                nc.vector.tensor_mul(e0, ru0[:, 64:128], d0)
                nc.vector.tensor_add(h01[:, 0:64], n0, e0)

                # layer1
                for m in range(8):
                    for k in range(KH):
                        nc.tensor.matmul(ps_c[:, 16 * m:16 * m + 16],
                                         wih1_sb[:, k, m, :],
                                         h01[:, 16 * k:16 * k + 16],
                                         start=(k == 0), stop=False)
                    for k in range(KH):
                        nc.tensor.matmul(ps_c[:, 16 * m:16 * m + 16],
                                         whh1_sb[:, k, m, :],
                                         h01[:, 64 + 16 * k:64 + 16 * k + 16],
                                         start=False, stop=(k == KH - 1))
                for m in range(8, MT):
                    mm = m - 8
                    for k in range(KH):
                        nc.tensor.matmul(ps_c[:, 128 + 16 * mm:128 + 16 * mm + 16],
                                         wih1_sb[:, k, m, :],
                                         h01[:, 16 * k:16 * k + 16],
                                         start=(k == 0), stop=(k == KH - 1))
                    for k in range(KH):
                        nc.tensor.matmul(ps_c[:, 192 + 16 * mm:192 + 16 * mm + 16],
                                         whh1_sb[:, k, m, :],
                                         h01[:, 64 + 16 * k:64 + 16 * k + 16],
                                         start=(k == 0), stop=(k == KH - 1))

                s_ru1 = wp.tile([128, 128], dt, tag="s_ru1")
                nc.vector.tensor_add(s_ru1, ps_c[:, 0:128], b1ru)
                ru1 = wp.tile([128, 128], dt, tag="ru1")
                nc.scalar.activation(ru1, s_ru1, AF.Sigmoid)
                hn1 = wp.tile([128, 64], dt, tag="hn1")
                nc.vector.tensor_add(hn1, ps_c[:, 192:256], bhh1n)
                rhn1 = wp.tile([128, 64], dt, tag="rhn1")
                nc.vector.tensor_mul(rhn1, ru1[:, 0:64], hn1)
                s_n1 = wp.tile([128, 64], dt, tag="s_n1")
                nc.vector.tensor_add(s_n1, ps_c[:, 128:192], bih1n)
                nin1 = wp.tile([128, 64], dt, tag="nin1")
                nc.vector.tensor_add(nin1, s_n1, rhn1)
                n1 = wp.tile([128, 64], dt, tag="n1")
                nc.scalar.activation(n1, nin1, AF.Tanh)
                d1 = wp.tile([128, 64], dt, tag="d1")
                nc.vector.tensor_sub(d1, h01[:, 64:128], n1)
                e1 = wp.tile([128, 64], dt, tag="e1")
                nc.vector.tensor_mul(e1, ru1[:, 64:128], d1)
                nc.vector.tensor_add(h01[:, 64:128], n1, e1)

                nc.sync.dma_start(out=scr_c[ds(s * 128, 128)],
                                  in_=h01[:, 64:128])

            nc.sync.dma_start(out=h_out, in_=h01)
    return nc


def build_heads(nc, T_):
    """token-major heads over scr [T_*128, 64]."""
    import concourse.mybir as mybir
    from concourse.tile import TileContext
    dt = mybir.dt.float32
    AF = mybir.ActivationFunctionType
    OP = mybir.AluOpType
    ntt = T_ * NB // 128

    scr = _din(nc, "scr", (T_ * 128, 64), dt)
    hW = _din(nc, "hW", (KH, 128, 512), dt)
    mub_s = _din(nc, "mub_s", (128, F), dt)
    lsb_s = _din(nc, "lsb_s", (128, F), dt)
    loadb_s = _din(nc, "loadb_s", (128, 256), dt)
    fh = _din(nc, "fh", (ntt, 128, NK), dt)

    cond_d = _dout(nc, "cond_d", (ntt, 128, F), dt)
    sig_d = _dout(nc, "sig_d", (ntt, 128, F), dt)
    load_d = _dout(nc, "load_d", (ntt, 128, 256), dt)

    scr3 = scr.rearrange("(t p) c -> t p c", p=128)
    with TileContext(nc) as tc:
        with tc.tile_pool(name="const", bufs=1) as cp, \
             tc.tile_pool(name="hps", bufs=4, space="PSUM") as hp, \
             tc.tile_pool(name="hwork", bufs=4) as hw, \
             tc.tile_pool(name="hin", bufs=6) as hi:
            def csb(src, shape, tag):
                t = cp.tile(list(shape), dt, tag=tag)
                nc.sync.dma_start(out=t, in_=src)
                return t

            hW_sb = csb(hW.rearrange("k p c -> p k c"), (128, KH, 512), "hW")
            mub = csb(mub_s, (128, F), "mub")
            lsb = csb(lsb_s, (128, F), "lsb")
            loadb = csb(loadb_s, (128, 256), "loadb")

            for tt in range(ntt):
                lh = []
                for k in range(KH):
                    t = hi.tile([128, 8, NB], dt, tag=f"lh{k}")
                    src = scr3[8 * tt:8 * tt + 8, :, 16 * k:16 * k + 16]
                    nc.sync.dma_start(out=t,
                                      in_=src.rearrange("t p b -> p t b"))
                    lh.append(t)
                ft = hi.tile([128, NK], dt, tag="ft")
                nc.sync.dma_start(out=ft, in_=fh[tt])

                hps = hp.tile([128, 512], dt, tag="hps")
                for k in range(KH):
                    nc.tensor.matmul(hps, lh[k].rearrange("p t b -> p (t b)"),
                                     hW_sb[:, k, :],
                                     start=(k == 0), stop=(k == KH - 1))

                mu_sb = hw.tile([128, F], dt, tag="mu_sb")
                nc.scalar.copy(mu_sb, hps[:, 0:128])
                t1 = hw.tile([128, F], dt, tag="t1")
                nc.vector.scalar_tensor_tensor(
                    t1, hps[:, 256:384], ft[:, 0:1], mu_sb,
                    op0=OP.mult, op1=OP.add)
                c0 = hw.tile([128, F], dt, tag="c0")
                nc.vector.scalar_tensor_tensor(
                    c0, hps[:, 384:512], ft[:, 1:2], t1,
                    op0=OP.mult, op1=OP.add)
                cnd = hw.tile([128, F], dt, tag="cnd")
                nc.vector.tensor_add(cnd, c0, mub)
                nc.sync.dma_start(out=cond_d[tt], in_=cnd)

                lss = hw.tile([128, F], dt, tag="lss")
                nc.vector.tensor_add(lss, hps[:, 128:256], lsb)
                lsc = hw.tile([128, F], dt, tag="lsc")
                nc.vector.tensor_scalar(
                    out=lsc, in0=lss, scalar1=float(MAX_LS),
                    scalar2=float(MIN_LS), op0=OP.min, op1=OP.max)
                sg = hw.tile([128, F], dt, tag="sg")
                nc.scalar.activation(sg, lsc, AF.Exp)
                nc.sync.dma_start(out=sig_d[tt], in_=sg)

                ld = hw.tile([128, 256], dt, tag="ld")
                nc.vector.tensor_add(ld, hps[:, 256:512], loadb)
                nc.sync.dma_start(out=load_d[tt], in_=ld)
    return nc


# ---------------- jitted runner around the bass custom-call ----------------

def make_runner(nc):
    import jax
    import concourse.mybir as mybir
    from jax.sharding import Mesh, PartitionSpec
    try:
        from jax.experimental.shard_map import shard_map
    except ImportError:
        from jax.shard_map import shard_map
    from concourse.bass2jax import (_bass_exec_p, install_neuronx_cc_hook,
                                    partition_id_tensor)

    install_neuronx_cc_hook()
    part_name = (nc.partition_id_tensor.name
                 if nc.partition_id_tensor is not None else None)
    in_names, out_names, out_avals, out_shapes = [], [], [], []
    for alloc in nc.m.functions[0].allocations:
        if not isinstance(alloc, mybir.MemoryLocationSet):
            continue
        name = alloc.memorylocations[0].name
        if alloc.kind == "ExternalInput":
            if name == part_name:
                continue
            in_names.append(name)
        elif alloc.kind == "ExternalOutput":
            out_names.append(name)
            shape = tuple(alloc.tensor_shape)
            dtype = mybir.dt.np(alloc.dtype)
            out_avals.append(jax.core.ShapedArray(shape, dtype))
            out_shapes.append((shape, dtype))
    n_params = len(in_names)
    all_names = list(in_names + out_names)
    if part_name is not None:
        all_names.append(part_name)
    all_names = tuple(all_names)

    def _body(*args):
        operands = list(args)
        if part_name is not None:
            operands.append(partition_id_tensor())
        outs = _bass_exec_p.bind(
            *operands, out_avals=tuple(out_avals), in_names=all_names,
            out_names=tuple(out_names), lowering_input_output_aliases=(),
            sim_require_finite=True, sim_require_nnan=True, nc=nc)
        return tuple(outs)

    mesh = Mesh(np.asarray(jax.devices()[:NCORES]), ("core",))
    f = jax.jit(shard_map(
        _body, mesh=mesh,
        in_specs=(PartitionSpec("core"),) * (n_params + len(out_names)),
        out_specs=(PartitionSpec("core"),) * len(out_names),
        check_rep=False))

    def run(named_inputs):
        import jax.numpy as jnp
        args = [named_inputs[n] for n in in_names]
        zeros = [jnp.zeros((NCORES * s[0],) + s[1:], d) for s, d in out_shapes]
        outs = f(*args, *zeros)
        return dict(zip(out_names, outs))

    return run


def _stack(maps, key):
    return np.concatenate([m[key] for m in maps], axis=0)


def _run(iv, T_):
    import jax
    import jax.numpy as jnp

    nchunks = T_ // TC
    maps = [_prep_core(iv, c, T_) for c in range(NCORES)]
    dp = {k: jax.device_put(_stack(maps, k),
                            jax.sharding.NamedSharding(
                                jax.sharding.Mesh(
                                    np.asarray(jax.devices()[:NCORES]),
                                    ("core",)),
                                jax.sharding.PartitionSpec("core")))
          for k in maps[0]}

    def _fin(nc):
        nc.finalize()
        return nc

    run_init = make_runner(_fin(build_init(_mk_nc())))
    run_chunk = make_runner(_fin(build_chunk(_mk_nc())))
    run_heads = make_runner(_fin(build_heads(_mk_nc(), T_)))

    o = run_init({"zT": dp["zT"], "fcT": dp["fcT"], "wzT": dp["wzT"],
                  "fcb_s": dp["fcb_s"], "bih0_s": dp["bih0_s"],
                  "bhh0_ru_s": dp["bhh0_ru_s"], "bih1_ru_s": dp["bih1_ru_s"],
                  "bhh1_ru_s": dp["bhh1_ru_s"]})
    h = o["h01_d"]
    base = {"gizb_ru_d": o["gizb_ru_d"], "gizb_n_d": o["gizb_n_d"],
            "b1ru_d": o["b1ru_d"], "bih1_n_s": dp["bih1_n_s"],
            "bhh0_n_s": dp["bhh0_n_s"], "bhh1_n_s": dp["bhh1_n_s"],
            "wxT": dp["wxT"], "whh0T": dp["whh0T"], "wih1T": dp["wih1T"],
            "whh1T": dp["whh1T"]}
    # x chunks: global [8*T_*128, 16] -> per chunk [8*TC*128, 16]
    xg = dp["xT"].reshape(NCORES, T_ * 128, NB)
    scrs = []
    for ci in range(nchunks):
        xc = xg[:, ci * TC * 128:(ci + 1) * TC * 128].reshape(-1, NB)
        o = run_chunk({**base, "h_in": h, "x_c": xc})
        h = o["h_out"]
        scrs.append(o["scr_c"])
    # assemble scr: per-chunk [8, TC*128, 64] -> [8, T_*128, 64]
    scr = jnp.concatenate(
        [s.reshape(NCORES, TC * 128, 64) for s in scrs], axis=1
    ).reshape(NCORES * T_ * 128, 64)

    o = run_heads({"scr": scr, "hW": dp["hW"], "mub_s": dp["mub_s"],
                   "lsb_s": dp["lsb_s"], "loadb_s": dp["loadb_s"],
                   "fh": dp["fh"]})
    ntt = T_ * NB // 128
    conds, sigs, loads = [], [], []
    cd = np.asarray(o["cond_d"]).reshape(NCORES, ntt, 8, NB, F)
    sd = np.asarray(o["sig_d"]).reshape(NCORES, ntt, 8, NB, F)
    ldd = np.asarray(o["load_d"]).reshape(NCORES, ntt, 8, NB, NK, F)
    for c in range(NCORES):
        conds.append(cd[c].transpose(2, 0, 1, 3).reshape(NB, T_, F))
        sigs.append(sd[c].transpose(2, 0, 1, 3).reshape(NB, T_, F))
        loads.append(ldd[c].transpose(2, 0, 1, 4, 3).reshape(NB, T_, F, NK))
    return (np.concatenate(conds, 0), np.concatenate(sigs, 0),
            np.concatenate(loads, 0))


def kernel(**inputs):
    iv = {k: np.asarray(v, dtype=F32) for k, v in inputs.items()}
    return _run(iv, T)
